# revision 33
# baseline (speedup 1.0000x reference)
"""Causal self-attention (B=2, T=2048, C=1024, 16 heads) on 8 trn2 NeuronCores.

Sharding: tensor-parallel over heads (4-way) x data-parallel over batch (2-way).
Core r handles batch dp = r // 4 and heads [4*tp, 4*tp+4) where tp = r % 4.

v4 design (ACT-engine-bound problem: softmax exp is ~58us/core minimum).
All PSUM lives in ONE pool of four double-bank [128,2,512] tiles (P01, P23,
P45, P67) that are reused phase to phase; the tile framework's subtile WAR
tracking sequences the reuse:
  P01: m=0 q psum (n0,n1) -> score tile for heads j=0,1 -> phase-3 psum
  P23: m=0 q psum (n2,n3) -> score tile for heads j=2,3 -> phase-3 psum
  P45: m=0 k psum (n0,n1) -> m=1 q/k psum (n-outer)   -> PV accum j=0,1
  P67: m=0 k psum (n2,n3) -> warm-up + v psum          -> PV accum j=2,3
Phase 2 ping-pongs scores/exp between P01 and P23: scores of chunk i+1
overwrite P01 while the exp of chunk i's P23 half still runs, so the ACT
exp stream never waits on the PE (the ACT->PE->ACT semaphore round trip is
~1us; with a single score tile it lands on every op).
The exp is one [128,2,*] ACTIVATE per half (80 total); the causal mask is a
single [128,4,128] bf16 multiply on the diagonal block only.  PV uses the
[v|1] ones-column trick (M=65): psum row 64 accumulates the softmax
denominator.  Window close = DVE copy [65,512] + den staged to partition 0
(reciprocal_approx_fast mis-reads partition-offset inputs) + reciprocal +
gpsimd broadcast + DVE normalize into yT.  Phase 3 is pn-outer/cc-inner so
psum->SBUF bf16 copies (alternating Scalar/Vector) pipeline with the MMs;
output is DMA'd as bf16 row-blocks (host sums the 4 TP partials + bias).
q/k biases are DVE tensor_scalar ops (keeps ACT free for exp); PE warm-up
matmuls + a tiny warm exp at t0 hold the HAM clock gate at 8/8 and preload
the exp table during the initial DMA window.
"""

import numpy as np

B, T, C = 2, 2048, 1024
NH, HD = 16, 64
NCORES, TPG = 8, 4          # 4-way tensor parallel x 2-way data parallel
HPC = NH // TPG             # heads per core (4)
DH = HPC * HD               # per-core head channels (256)
KC = C // 128               # contraction chunks over C (8)
NT4 = T // 512              # 512-wide q windows (4)
NT = T // 128               # 128-wide k/t tiles (16)

_PROG = None
TRACE = False
DEBUG = False
LAST_RESULTS = None


def _build():
    import concourse.bacc as bacc
    import concourse.mybir as mybir
    from concourse import tile

    F32 = mybir.dt.float32
    BF16 = mybir.dt.bfloat16
    AF = mybir.ActivationFunctionType

    nc = bacc.Bacc("TRN2", target_bir_lowering=False, debug=False,
                   num_devices=NCORES)

    xT = nc.dram_tensor("xT", [C, T], BF16, kind="ExternalInput").ap()
    wq4 = nc.dram_tensor("wq4", [128, KC * DH], BF16, kind="ExternalInput").ap()
    wk4 = nc.dram_tensor("wk4", [128, KC * DH], BF16, kind="ExternalInput").ap()
    wv4 = nc.dram_tensor("wv4", [128, KC * DH], BF16, kind="ExternalInput").ap()
    wp4 = nc.dram_tensor("wp4", [128, 2 * C], BF16, kind="ExternalInput").ap()
    bq2 = nc.dram_tensor("bq2", [128, 2], F32, kind="ExternalInput").ap()
    bk2 = nc.dram_tensor("bk2", [128, 2], F32, kind="ExternalInput").ap()
    bv1 = nc.dram_tensor("bv1", [1, DH], F32, kind="ExternalInput").ap()
    mask_d = nc.dram_tensor("mask_d", [128, 4 * 128], BF16, kind="ExternalInput").ap()
    yout = nc.dram_tensor("yout", [C, T], BF16, kind="ExternalOutput").ap()
    if DEBUG:
        qT_d = nc.dram_tensor("qT_d", [128, 2 * T], BF16, kind="ExternalOutput").ap()
        kT_d = nc.dram_tensor("kT_d", [128, 2 * T], BF16, kind="ExternalOutput").ap()
        v4_d = nc.dram_tensor("v4_d", [128, NT * HPC * (HD + 1)], BF16,
                              kind="ExternalOutput").ap()
        yT_d = nc.dram_tensor("yT_d", [128, 2 * T], BF16, kind="ExternalOutput").ap()
        yh_d = nc.dram_tensor("yh_d", [65, 4 * 512], F32, kind="ExternalOutput").ap()
        st_d = nc.dram_tensor("st_d", [128, 4 * 512], BF16, kind="ExternalOutput").ap()

    with tile.TileContext(nc) as tc:
        with tc.tile_pool(name="const", bufs=1) as constp, \
             tc.tile_pool(name="qkv", bufs=1) as qkvp, \
             tc.tile_pool(name="xt", bufs=1) as xtp, \
             tc.tile_pool(name="strip", bufs=15) as stripp, \
             tc.tile_pool(name="rec", bufs=1) as recp, \
             tc.tile_pool(name="outp", bufs=2) as outp, \
             tc.tile_pool(name="ps", bufs=1, space="PSUM") as psp:

            wq_sb = constp.tile([128, KC, DH], BF16)
            wk_sb = constp.tile([128, KC, DH], BF16)
            wv_sb = constp.tile([128, KC, DH], BF16)
            wp_sb = constp.tile([128, 2, C], BF16)
            bq_sb = constp.tile([128, 2], F32)
            bk_sb = constp.tile([128, 2], F32)
            bv_sb = constp.tile([1, DH], F32)
            bv_bc = constp.tile([128, DH], F32)
            mask_sb = constp.tile([128, 4, 128], BF16)
            warm_in = constp.tile([128, 96], BF16)
            warm_out = constp.tile([1, 64], F32)
            ones64 = constp.tile([1, 64], F32)

            qT_sb = qkvp.tile([128, 2, T], BF16)   # [64*(h%2)+d, h//2, t]
            kT_sb = qkvp.tile([128, 2, T], BF16)
            v4 = qkvp.tile([128, NT, HPC, HD + 1], BF16)  # [t%128, t//128, h, d|1]
            yT_sb = qkvp.tile([128, 2, T], BF16)
            xT_sb = xtp.tile([128, KC, T], BF16)

            P01 = psp.tile([128, 2, 512], F32, tag="p01", name="P01")
            P23 = psp.tile([128, 2, 512], F32, tag="p23", name="P23")
            P45 = psp.tile([128, 2, 512], F32, tag="p45", name="P45")
            P67 = psp.tile([128, 2, 512], F32, tag="p67", name="P67")
            # m0 psum slice for (w, n):  q -> P01/P23, k -> P45/P67
            m0ps = {(0, 0): P01[:, 0, :], (0, 1): P01[:, 1, :],
                    (0, 2): P23[:, 0, :], (0, 3): P23[:, 1, :],
                    (1, 0): P45[:, 0, :], (1, 1): P45[:, 1, :],
                    (1, 2): P67[:, 0, :], (1, 3): P67[:, 1, :]}
            psy = [P45[:, 0, :], P45[:, 1, :], P67[:, 0, :], P67[:, 1, :]]

            # ---------------- t0: DMA issues, warm-ups ----------------
            nc.gpsimd.memset(warm_in[:], 0.0078125)
            nc.gpsimd.memset(ones64[:], 1.0)
            nc.gpsimd.dma_start(out=wq_sb[:].rearrange("p c m -> p (c m)"), in_=wq4)
            nc.gpsimd.dma_start(out=wk_sb[:].rearrange("p c m -> p (c m)"), in_=wk4)
            nc.gpsimd.dma_start(out=bq_sb[:], in_=bq2)
            nc.gpsimd.dma_start(out=bk_sb[:], in_=bk2)
            nc.gpsimd.dma_start(out=wv_sb[:].rearrange("p c m -> p (c m)"), in_=wv4)
            nc.gpsimd.dma_start(out=bv_sb[:], in_=bv1)
            nc.gpsimd.dma_start(out=mask_sb[:].rearrange("p a b -> p (a b)"), in_=mask_d)
            nc.gpsimd.dma_start(out=wp_sb[:].rearrange("p c m -> p (c m)"), in_=wp4)
            nc.gpsimd.partition_broadcast(bv_bc[:], bv_sb[:])
            nc.vector.memset(v4[:], 1.0)  # ones column survives; rest overwritten
            xTr = xT.rearrange("(c p) t -> p c t", p=128)
            for c in range(KC):
                nc.sync.dma_start(out=xT_sb[:, c, :], in_=xTr[:, c, :])
            # preload the exp table set while ACT is idle
            nc.scalar.activation(warm_out[:], warm_in[0:1, 0:64], AF.Exp)
            # keep the PE busy through the DMA window so HAM sits at 8/8
            for _ in range(52):
                nc.tensor.matmul(P67[0:96, 1, 0:96], lhsT=warm_in[:, 0:96],
                                 rhs=warm_in[:, 0:96], start=True, stop=True)

            def bias_dve(w, n, m):
                b_sb = bq_sb if w == 0 else bk_sb
                dst = (qT_sb if w == 0 else kT_sb)[:, m, 512 * n:512 * (n + 1)]
                src = m0ps[(w, n)] if m == 0 else P45[:, w, :]
                with nc.allow_low_precision(reason="bf16 out"):
                    nc.vector.tensor_scalar_add(dst, src, b_sb[:, m:m + 1])

            # ---------------- phase 1a: q/k m=0 (c-outer, 8 banks) ----------
            for c in range(KC):
                for w, w_sb in ((0, wq_sb), (1, wk_sb)):
                    for n in range(NT4):
                        nc.tensor.matmul(
                            m0ps[(w, n)], lhsT=w_sb[:, c, 0:128],
                            rhs=xT_sb[:, c, 512 * n:512 * (n + 1)],
                            start=(c == 0), stop=(c == KC - 1))
            # biases that gate the m=1 psum (P45) and the score tiles (P01/P23)
            for w, n in ((1, 0), (1, 1), (0, 0), (0, 1), (0, 2), (0, 3)):
                bias_dve(w, n, 0)

            # ---------------- phases 1b + 2 ----------------
            stream = [(n4, c) for n4 in range(NT4) for c in range(4 * (n4 + 1))]
            pvq = []
            normq = []
            close_si = [0]
            win = {}
            si = 0  # next stream chunk to emit
            last_stp = [None]

            def emit_chunk():
                nonlocal si
                n4, c = stream[si]
                o = c - 4 * n4
                qo = 128 * o if o > 0 else 0
                stp = stripp.tile([128, 4, 512], BF16, tag="stp", name="stp")
                for m, Ps in ((0, P01), (1, P23)):
                    for hh in range(2):
                        nc.tensor.matmul(
                            Ps[:, hh, qo:512],
                            lhsT=kT_sb[64 * hh:64 * (hh + 1), m,
                                       128 * c:128 * (c + 1)],
                            rhs=qT_sb[64 * hh:64 * (hh + 1), m,
                                      512 * n4 + qo:512 * (n4 + 1)],
                            start=True, stop=True, tile_position=(64 * hh, 0))
                    # per-half exp: scores of the next chunk overwrite P01
                    # while this chunk's P23 exp still runs
                    nc.scalar.activation(stp[:, 2 * m:2 * m + 2, qo:512],
                                         Ps[:, :, qo:512], AF.Exp)
                if o >= 0:
                    with nc.allow_low_precision(reason="0/1 mask"):
                        nc.vector.tensor_mul(stp[:, :, qo:qo + 128],
                                             stp[:, :, qo:qo + 128],
                                             mask_sb[:])
                pvq.append((n4, c, qo, stp))
                last_stp[0] = stp
                si += 1

            def emit_m1_half(n, half):
                for c in range(4 * half, 4 * half + 4):
                    for w, w_sb in ((0, wq_sb), (1, wk_sb)):
                        nc.tensor.matmul(
                            P45[:, w, :], lhsT=w_sb[:, c, 128:256],
                            rhs=xT_sb[:, c, 512 * n:512 * (n + 1)],
                            start=(c == 0), stop=(c == KC - 1))
                if half == 1:
                    for w in range(2):
                        bias_dve(w, n, 1)

            def emit_v_tile(t):
                vp = P67[:, 0, 0:DH]
                for c in range(KC):
                    nc.tensor.matmul(
                        vp, lhsT=xT_sb[:, c, 128 * t:128 * (t + 1)],
                        rhs=wv_sb[:, c, :], start=(c == 0), stop=(c == KC - 1))
                with nc.allow_low_precision(reason="f32r bits == f32 bits"):
                    nc.vector.tensor_add(
                        v4[:, t, :, 0:HD],
                        vp.rearrange("p (h d) -> p h d", h=HPC),
                        bv_bc[:].rearrange("p (h d) -> p h d", h=HPC))

            # m=1 n-blocks, v tiles, and early chunks interleaved so the exp
            # stream starts while projections still run on the PE.
            emit_m1_half(0, 0)
            emit_m1_half(0, 1)
            emit_chunk()                      # 0 (needs only n0 q/k)
            emit_m1_half(1, 0)
            emit_m1_half(1, 1)
            emit_chunk()                      # 1
            bias_dve(1, 2, 0)                 # frees P67 slice 0 for v psum
            bias_dve(1, 3, 0)
            emit_chunk()                      # 2
            emit_m1_half(2, 0)
            emit_m1_half(2, 1)
            emit_chunk()                      # 3
            for t in (0, 1):
                emit_v_tile(t)
            emit_chunk()                      # 4
            for t in (2, 3):
                emit_v_tile(t)
            emit_chunk()                      # 5
            emit_m1_half(3, 0)
            emit_m1_half(3, 1)
            emit_chunk()                      # 6
            for t in (4, 5):
                emit_v_tile(t)
            emit_chunk()                      # 7
            for t in (6, 7):
                emit_v_tile(t)
            emit_chunk()                      # 8
            for t in (8, 9):
                emit_v_tile(t)
            emit_chunk()                      # 9
            for t in (10, 11):
                emit_v_tile(t)
            emit_chunk()                      # 10
            for t in (12, 13):
                emit_v_tile(t)
            emit_chunk()                      # 11
            for t in (14, 15):
                emit_v_tile(t)
            emit_chunk()                      # 12

            def open_window(n4):
                win[n4] = dict(
                    yh=[recp.tile([65, 512], F32, tag=f"yh{j}", name="yh")
                        for j in range(4)],
                    dn=[recp.tile([1, 512], F32, tag=f"dn{j}", name="dn")
                        for j in range(4)],
                    rr=[recp.tile([1, 512], F32, tag=f"rr{j}", name="rr")
                        for j in range(4)],
                )

            def close_window(n4):
                st = win[n4]
                rbcs = []
                for j in range(4):
                    with nc.allow_low_precision(reason="f32 bits"):
                        nc.vector.tensor_copy(st["yh"][j][:], psy[j][0:65, :])
                    # recip_approx_fast mis-reads partition-offset inputs --
                    # stage the den row at partition 0 first
                    with nc.allow_low_precision(reason="f32 bits"):
                        nc.vector.tensor_copy(st["dn"][j][:],
                                              st["yh"][j][64:65, :])
                    nc.vector.reciprocal_approx_fast(st["rr"][j][:],
                                                     st["dn"][j][:])
                    # emit the broadcast right away so gpsimd overlaps the
                    # remaining DVE copies (the muls stay in a second loop so
                    # they don't block the DVE queue on gpsimd latency)
                    rbc = recp.tile([64, 512], F32, tag="rbc", bufs=4,
                                    name="rbc")
                    nc.gpsimd.partition_broadcast(rbc[:], st["rr"][j][:])
                    rbcs.append(rbc)
                for j in range(4):
                    m, hh = j // 2, j % 2
                    with nc.allow_low_precision(reason="bf16 out"):
                        nc.vector.tensor_mul(
                            yT_sb[64 * hh:64 * (hh + 1), m,
                                  512 * n4:512 * (n4 + 1)],
                            st["yh"][j][0:64, :], rbcs[j][:])

            def pv():
                n4, c, qo, stp = pvq.pop(0)
                if c == 0:
                    open_window(n4)
                nch = 4 * (n4 + 1)
                for j in range(4):
                    nc.tensor.matmul(
                        psy[j][0:65, qo:512],
                        lhsT=v4[:, c, j, :],
                        rhs=stp[:, j, qo:512],
                        start=(c == 0), stop=(c == nch - 1))
                if c == nch - 1:
                    close_window(n4)

            while si < len(stream):
                emit_chunk()
                npop = 2 if (len(pvq) > 4 and si % 2 == 0) else 1
                for _ in range(min(npop, len(pvq))):
                    pv()
            while pvq:
                pv()

            if DEBUG:
                for j in range(4):
                    nc.sync.dma_start(out=yh_d[:, 512 * j:512 * (j + 1)],
                                      in_=win[3]["yh"][j][:])

            # ---------------- phase 3: out-projection ----------------
            ph3ps = [P01[:, 0, :], P01[:, 1, :], P23[:, 0, :], P23[:, 1, :]]
            for mo in range(8):
                oc = outp.tile([128, T], BF16, tag="oc", name="oc")
                for pn in range(NT4):
                    if mo == 0 and pn == 3:
                        # keep-warm MMs into the dead psy bank: they span the
                        # win3 normalize wait so HAM stays at 8/8, and nothing
                        # in phase 3 reads P45 so they delay no real work
                        for _ in range(20):
                            nc.tensor.matmul(P45[0:96, 0, :],
                                             lhsT=warm_in[:, 0:96],
                                             rhs=xT_sb[:, 0, 0:512],
                                             start=True, stop=True)
                    for cc in range(2):
                        nc.tensor.matmul(
                            ph3ps[pn],
                            lhsT=wp_sb[:, cc, 128 * mo:128 * (mo + 1)],
                            rhs=yT_sb[:, cc, 512 * pn:512 * (pn + 1)],
                            start=(cc == 0), stop=(cc == 1))
                    dst = oc[:, 512 * pn:512 * (pn + 1)]
                    if pn % 2 == 0:
                        nc.scalar.activation(dst, ph3ps[pn], AF.Copy)
                    else:
                        with nc.allow_low_precision(reason="bf16 out"):
                            nc.vector.tensor_copy(dst, ph3ps[pn])
                eng = nc.sync if mo % 2 == 0 else nc.gpsimd
                eng.dma_start(out=yout[128 * mo:128 * (mo + 1), :], in_=oc[:])

            if DEBUG:
                nc.sync.dma_start(out=qT_d, in_=qT_sb[:].rearrange("p m t -> p (m t)"))
                nc.sync.dma_start(out=kT_d, in_=kT_sb[:].rearrange("p m t -> p (m t)"))
                nc.sync.dma_start(out=v4_d, in_=v4[:].rearrange("p a b c -> p (a b c)"))
                nc.sync.dma_start(out=yT_d, in_=yT_sb[:].rearrange("p m t -> p (m t)"))
                nc.sync.dma_start(out=st_d, in_=last_stp[0][:].rearrange("p a b -> p (a b)"))

    nc.compile()
    return nc


def _bf16():
    import ml_dtypes
    return ml_dtypes.bfloat16


def _rearr(w2):
    # [KC*128, M] -> [128, KC*M] so the SBUF load is one contiguous DMA
    m = w2.shape[1]
    return np.ascontiguousarray(
        w2.reshape(KC, 128, m).transpose(1, 0, 2).reshape(128, KC * m))


def kernel(x, Wq, bq, Wk, bk, Wv, bv, Wp, bp):
    global _PROG, LAST_RESULTS
    from concourse.bass_utils import run_bass_kernel_spmd

    x = np.asarray(x, np.float32)
    Wq = np.asarray(Wq, np.float32)
    bq = np.asarray(bq, np.float32)
    Wk = np.asarray(Wk, np.float32)
    bk = np.asarray(bk, np.float32)
    Wv = np.asarray(Wv, np.float32)
    bv = np.asarray(bv, np.float32)
    Wp = np.asarray(Wp, np.float32)
    bp = np.asarray(bp, np.float32)

    if _PROG is None:
        _PROG = _build()
    nc = _PROG

    scale = np.float32(1.0 / np.sqrt(HD))
    k_i = np.arange(128)[:, None]
    q_i = np.arange(128)[None, :]
    tri = (q_i >= k_i).astype(np.float32)           # [k, q] lower-tri in S^T
    mask4 = np.broadcast_to(tri[:, None, :], (128, 4, 128)).reshape(128, 512)
    mask_b = np.ascontiguousarray(mask4).astype(_bf16())

    in_maps = []
    for r in range(NCORES):
        tp, dp = r % TPG, r // TPG
        sl = slice(DH * tp, DH * (tp + 1))
        in_maps.append({
            "xT": np.ascontiguousarray(x[dp].T).astype(_bf16()),
            "wq4": _rearr((Wq[sl] * scale).T).astype(_bf16()),
            "wk4": _rearr(Wk[sl].T).astype(_bf16()),
            "wv4": _rearr(Wv[sl].T).astype(_bf16()),
            "wp4": np.ascontiguousarray(
                Wp[:, sl].T.reshape(2, 128, C).transpose(1, 0, 2)
                .reshape(128, 2 * C)).astype(_bf16()),
            "bq2": np.ascontiguousarray((bq[sl] * scale).reshape(2, 128).T),
            "bk2": np.ascontiguousarray(bk[sl].reshape(2, 128).T),
            "bv1": bv[sl].reshape(1, DH).copy(),
            "mask_d": mask_b,
        })

    res = run_bass_kernel_spmd(nc, in_maps, core_ids=list(range(NCORES)),
                               trace=TRACE)
    LAST_RESULTS = res

    out = np.empty((B, T, C), np.float32)
    for dp in range(B):
        acc = res.results[TPG * dp]["yout"].astype(np.float32)
        for tp in range(1, TPG):
            acc += res.results[TPG * dp + tp]["yout"].astype(np.float32)
        out[dp] = acc.T + bp
    return out


# revision 35
# speedup vs baseline: 1.0277x; 1.0277x over previous
"""Causal self-attention (B=2, T=2048, C=1024, 16 heads) on 8 trn2 NeuronCores.

Sharding: tensor-parallel over heads (4-way) x data-parallel over batch (2-way).
Core r handles batch dp = r // 4 and heads [4*tp, 4*tp+4) where tp = r % 4.

v4 design (ACT-engine-bound problem: softmax exp is ~58us/core minimum).
All PSUM lives in ONE pool of four double-bank [128,2,512] tiles (P01, P23,
P45, P67) that are reused phase to phase; the tile framework's subtile WAR
tracking sequences the reuse:
  P01: m=0 q psum (n0,n1) -> score tile for heads j=0,1 -> phase-3 psum
  P23: m=0 q psum (n2,n3) -> score tile for heads j=2,3 -> phase-3 psum
  P45: m=0 k psum (n0,n1) -> m=1 q/k psum (n-outer)   -> PV accum j=0,1
  P67: m=0 k psum (n2,n3) -> warm-up + v psum          -> PV accum j=2,3
Phase 2 ping-pongs scores/exp between P01 and P23: scores of chunk i+1
overwrite P01 while the exp of chunk i's P23 half still runs, so the ACT
exp stream never waits on the PE (the ACT->PE->ACT semaphore round trip is
~1us; with a single score tile it lands on every op).
The exp is one [128,2,*] ACTIVATE per half (80 total); the causal mask is a
single [128,4,128] bf16 multiply on the diagonal block only.  PV uses the
[v|1] ones-column trick (M=65): psum row 64 accumulates the softmax
denominator.  Window close = DVE copy [65,512] + den staged to partition 0
(reciprocal_approx_fast mis-reads partition-offset inputs) + reciprocal +
gpsimd broadcast + DVE normalize into yT.  Phase 3 is pn-outer/cc-inner so
psum->SBUF bf16 copies (alternating Scalar/Vector) pipeline with the MMs;
output is DMA'd as bf16 row-blocks (host sums the 4 TP partials + bias).
q/k biases are DVE tensor_scalar ops (keeps ACT free for exp); PE warm-up
matmuls + a tiny warm exp at t0 hold the HAM clock gate at 8/8 and preload
the exp table during the initial DMA window.
"""

import numpy as np

B, T, C = 2, 2048, 1024
NH, HD = 16, 64
NCORES, TPG = 8, 4          # 4-way tensor parallel x 2-way data parallel
HPC = NH // TPG             # heads per core (4)
DH = HPC * HD               # per-core head channels (256)
KC = C // 128               # contraction chunks over C (8)
NT4 = T // 512              # 512-wide q windows (4)
NT = T // 128               # 128-wide k/t tiles (16)

_PROG = None
TRACE = False
DEBUG = False
LAST_RESULTS = None


def _build():
    import concourse.bacc as bacc
    import concourse.mybir as mybir
    from concourse import tile

    F32 = mybir.dt.float32
    BF16 = mybir.dt.bfloat16
    AF = mybir.ActivationFunctionType

    nc = bacc.Bacc("TRN2", target_bir_lowering=False, debug=False,
                   num_devices=NCORES)

    xT = nc.dram_tensor("xT", [C, T], BF16, kind="ExternalInput").ap()
    wq4 = nc.dram_tensor("wq4", [128, KC * DH], BF16, kind="ExternalInput").ap()
    wk4 = nc.dram_tensor("wk4", [128, KC * DH], BF16, kind="ExternalInput").ap()
    wv4 = nc.dram_tensor("wv4", [128, KC * DH], BF16, kind="ExternalInput").ap()
    wp4 = nc.dram_tensor("wp4", [128, 2 * C], BF16, kind="ExternalInput").ap()
    bq2 = nc.dram_tensor("bq2", [128, 2], F32, kind="ExternalInput").ap()
    bk2 = nc.dram_tensor("bk2", [128, 2], F32, kind="ExternalInput").ap()
    bv1 = nc.dram_tensor("bv1", [1, DH], F32, kind="ExternalInput").ap()
    mask_d = nc.dram_tensor("mask_d", [128, 4 * 128], BF16, kind="ExternalInput").ap()
    yout = nc.dram_tensor("yout", [C, T], BF16, kind="ExternalOutput").ap()
    if DEBUG:
        qT_d = nc.dram_tensor("qT_d", [128, 2 * T], BF16, kind="ExternalOutput").ap()
        kT_d = nc.dram_tensor("kT_d", [128, 2 * T], BF16, kind="ExternalOutput").ap()
        v4_d = nc.dram_tensor("v4_d", [128, NT * HPC * (HD + 1)], BF16,
                              kind="ExternalOutput").ap()
        yT_d = nc.dram_tensor("yT_d", [128, 2 * T], BF16, kind="ExternalOutput").ap()
        yh_d = nc.dram_tensor("yh_d", [65, 4 * 512], F32, kind="ExternalOutput").ap()
        st_d = nc.dram_tensor("st_d", [128, 4 * 512], BF16, kind="ExternalOutput").ap()

    with tile.TileContext(nc) as tc:
        with tc.tile_pool(name="const", bufs=1) as constp, \
             tc.tile_pool(name="qkv", bufs=1) as qkvp, \
             tc.tile_pool(name="xt", bufs=1) as xtp, \
             tc.tile_pool(name="strip", bufs=15) as stripp, \
             tc.tile_pool(name="rec", bufs=1) as recp, \
             tc.tile_pool(name="outp", bufs=2) as outp, \
             tc.tile_pool(name="ps", bufs=1, space="PSUM") as psp:

            wq_sb = constp.tile([128, KC, DH], BF16)
            wk_sb = constp.tile([128, KC, DH], BF16)
            wv_sb = constp.tile([128, KC, DH], BF16)
            wp_sb = constp.tile([128, 2, C], BF16)
            bq_sb = constp.tile([128, 2], F32)
            bk_sb = constp.tile([128, 2], F32)
            bv_sb = constp.tile([1, DH], F32)
            bv_bc = constp.tile([128, DH], F32)
            mask_sb = constp.tile([128, 4, 128], BF16)
            warm_in = constp.tile([128, 96], BF16)
            warm_out = constp.tile([1, 64], F32)
            ones64 = constp.tile([1, 64], F32)

            qT_sb = qkvp.tile([128, 2, T], BF16)   # [64*(h%2)+d, h//2, t]
            kT_sb = qkvp.tile([128, 2, T], BF16)
            v4 = qkvp.tile([128, NT, HPC, HD + 1], BF16)  # [t%128, t//128, h, d|1]
            yT_sb = qkvp.tile([128, 2, T], BF16)
            xT_sb = xtp.tile([128, KC, T], BF16)

            P01 = psp.tile([128, 2, 512], F32, tag="p01", name="P01")
            P23 = psp.tile([128, 2, 512], F32, tag="p23", name="P23")
            P45 = psp.tile([128, 2, 512], F32, tag="p45", name="P45")
            P67 = psp.tile([128, 2, 512], F32, tag="p67", name="P67")
            # m0 psum slice for (w, n):  q -> P01/P23, k -> P45/P67
            m0ps = {(0, 0): P01[:, 0, :], (0, 1): P01[:, 1, :],
                    (0, 2): P23[:, 0, :], (0, 3): P23[:, 1, :],
                    (1, 0): P45[:, 0, :], (1, 1): P45[:, 1, :],
                    (1, 2): P67[:, 0, :], (1, 3): P67[:, 1, :]}
            psy = [P45[:, 0, :], P45[:, 1, :], P67[:, 0, :], P67[:, 1, :]]

            # ---------------- t0: DMA issues, warm-ups ----------------
            nc.gpsimd.memset(warm_in[:], 0.0078125)
            nc.gpsimd.memset(ones64[:], 1.0)
            nc.gpsimd.dma_start(out=wq_sb[:].rearrange("p c m -> p (c m)"), in_=wq4)
            nc.gpsimd.dma_start(out=wk_sb[:].rearrange("p c m -> p (c m)"), in_=wk4)
            nc.gpsimd.dma_start(out=bq_sb[:], in_=bq2)
            nc.gpsimd.dma_start(out=bk_sb[:], in_=bk2)
            nc.gpsimd.dma_start(out=wv_sb[:].rearrange("p c m -> p (c m)"), in_=wv4)
            nc.gpsimd.dma_start(out=bv_sb[:], in_=bv1)
            nc.gpsimd.dma_start(out=mask_sb[:].rearrange("p a b -> p (a b)"), in_=mask_d)
            nc.gpsimd.dma_start(out=wp_sb[:].rearrange("p c m -> p (c m)"), in_=wp4)
            nc.gpsimd.partition_broadcast(bv_bc[:], bv_sb[:])
            nc.vector.memset(v4[:], 1.0)  # ones column survives; rest overwritten
            xTr = xT.rearrange("(c p) t -> p c t", p=128)
            for c in range(KC):
                nc.sync.dma_start(out=xT_sb[:, c, :], in_=xTr[:, c, :])
            # preload the exp table set while ACT is idle
            nc.scalar.activation(warm_out[:], warm_in[0:1, 0:64], AF.Exp)
            # keep the PE busy through the DMA window so HAM sits at 8/8
            for _ in range(52):
                nc.tensor.matmul(P67[0:96, 1, 0:96], lhsT=warm_in[:, 0:96],
                                 rhs=warm_in[:, 0:96], start=True, stop=True)

            def bias_dve(w, n, m):
                b_sb = bq_sb if w == 0 else bk_sb
                dst = (qT_sb if w == 0 else kT_sb)[:, m, 512 * n:512 * (n + 1)]
                src = m0ps[(w, n)] if m == 0 else P45[:, w, :]
                with nc.allow_low_precision(reason="bf16 out"):
                    nc.vector.tensor_scalar_add(dst, src, b_sb[:, m:m + 1])

            # ---------------- phase 1a: q/k m=0 (c-outer, 8 banks) ----------
            for c in range(KC):
                for w, w_sb in ((0, wq_sb), (1, wk_sb)):
                    for n in range(NT4):
                        nc.tensor.matmul(
                            m0ps[(w, n)], lhsT=w_sb[:, c, 0:128],
                            rhs=xT_sb[:, c, 512 * n:512 * (n + 1)],
                            start=(c == 0), stop=(c == KC - 1))
            # biases that gate the m=1 psum (P45) and the score tiles (P01/P23)
            for w, n in ((1, 0), (1, 1), (0, 0), (0, 1), (0, 2), (0, 3)):
                bias_dve(w, n, 0)

            # ---------------- phases 1b + 2 ----------------
            stream = [(n4, c) for n4 in range(NT4) for c in range(4 * (n4 + 1))]
            pvq = []
            normq = []
            close_si = [0]
            win = {}
            si = 0  # next stream chunk to emit
            last_stp = [None]

            def emit_chunk():
                nonlocal si
                n4, c = stream[si]
                o = c - 4 * n4
                qo = 128 * o if o > 0 else 0
                stp = stripp.tile([128, 4, 512], BF16, tag="stp", name="stp")
                for m, Ps in ((0, P01), (1, P23)):
                    for hh in range(2):
                        nc.tensor.matmul(
                            Ps[:, hh, qo:512],
                            lhsT=kT_sb[64 * hh:64 * (hh + 1), m,
                                       128 * c:128 * (c + 1)],
                            rhs=qT_sb[64 * hh:64 * (hh + 1), m,
                                      512 * n4 + qo:512 * (n4 + 1)],
                            start=True, stop=True, tile_position=(64 * hh, 0))
                    # per-half exp: scores of the next chunk overwrite P01
                    # while this chunk's P23 exp still runs
                    nc.scalar.activation(stp[:, 2 * m:2 * m + 2, qo:512],
                                         Ps[:, :, qo:512], AF.Exp)
                if o >= 0:
                    with nc.allow_low_precision(reason="0/1 mask"):
                        nc.vector.tensor_mul(stp[:, :, qo:qo + 128],
                                             stp[:, :, qo:qo + 128],
                                             mask_sb[:])
                pvq.append((n4, c, qo, stp))
                last_stp[0] = stp
                si += 1

            def emit_m1_half(n, half):
                for c in range(4 * half, 4 * half + 4):
                    for w, w_sb in ((0, wq_sb), (1, wk_sb)):
                        nc.tensor.matmul(
                            P45[:, w, :], lhsT=w_sb[:, c, 128:256],
                            rhs=xT_sb[:, c, 512 * n:512 * (n + 1)],
                            start=(c == 0), stop=(c == KC - 1))
                if half == 1:
                    for w in range(2):
                        bias_dve(w, n, 1)

            def emit_v_tile(t):
                vp = P67[:, 0, 0:DH]
                for c in range(KC):
                    nc.tensor.matmul(
                        vp, lhsT=xT_sb[:, c, 128 * t:128 * (t + 1)],
                        rhs=wv_sb[:, c, :], start=(c == 0), stop=(c == KC - 1))
                with nc.allow_low_precision(reason="f32r bits == f32 bits"):
                    nc.vector.tensor_add(
                        v4[:, t, :, 0:HD],
                        vp.rearrange("p (h d) -> p h d", h=HPC),
                        bv_bc[:].rearrange("p (h d) -> p h d", h=HPC))

            # m=1 n-blocks, v tiles, and early chunks interleaved so the exp
            # stream starts while projections still run on the PE.
            emit_m1_half(0, 0)
            emit_m1_half(0, 1)
            emit_chunk()                      # 0 (needs only n0 q/k)
            emit_m1_half(1, 0)
            emit_m1_half(1, 1)
            emit_chunk()                      # 1
            bias_dve(1, 2, 0)                 # frees P67 slice 0 for v psum
            bias_dve(1, 3, 0)
            emit_chunk()                      # 2
            emit_m1_half(2, 0)
            emit_m1_half(2, 1)
            emit_chunk()                      # 3
            for t in (0, 1):
                emit_v_tile(t)
            emit_chunk()                      # 4
            for t in (2, 3):
                emit_v_tile(t)
            emit_chunk()                      # 5
            emit_m1_half(3, 0)
            emit_m1_half(3, 1)
            emit_chunk()                      # 6
            for t in (4, 5):
                emit_v_tile(t)
            emit_chunk()                      # 7
            for t in (6, 7):
                emit_v_tile(t)
            emit_chunk()                      # 8
            for t in (8, 9):
                emit_v_tile(t)
            emit_chunk()                      # 9
            for t in (10, 11):
                emit_v_tile(t)
            emit_chunk()                      # 10
            for t in (12, 13):
                emit_v_tile(t)
            emit_chunk()                      # 11
            for t in (14, 15):
                emit_v_tile(t)
            emit_chunk()                      # 12

            def open_window(n4):
                win[n4] = dict(
                    yh=[recp.tile([65, 512], F32, tag=f"yh{j}", name="yh")
                        for j in range(4)],
                    dn=[recp.tile([1, 512], F32, tag=f"dn{j}", name="dn")
                        for j in range(4)],
                    rr=[recp.tile([1, 512], F32, tag=f"rr{j}", name="rr")
                        for j in range(4)],
                )

            def close_window(n4):
                st = win[n4]
                rbcs = []
                for j in range(4):
                    with nc.allow_low_precision(reason="f32 bits"):
                        nc.vector.tensor_copy(st["yh"][j][:], psy[j][0:65, :])
                    # recip_approx_fast mis-reads partition-offset inputs --
                    # stage the den row at partition 0 first
                    with nc.allow_low_precision(reason="f32 bits"):
                        nc.vector.tensor_copy(st["dn"][j][:],
                                              st["yh"][j][64:65, :])
                    nc.vector.reciprocal_approx_fast(st["rr"][j][:],
                                                     st["dn"][j][:])
                    if n4 == NT4 - 1:
                        continue  # win3 normalize runs in phase 3 (PE bcast)
                    # emit the broadcast right away so gpsimd overlaps the
                    # remaining DVE copies (the muls stay in a second loop so
                    # they don't block the DVE queue on gpsimd latency)
                    rbc = recp.tile([64, 512], F32, tag="rbc", bufs=4,
                                    name="rbc")
                    nc.gpsimd.partition_broadcast(rbc[:], st["rr"][j][:])
                    rbcs.append(rbc)
                if n4 == NT4 - 1:
                    return
                for j in range(4):
                    m, hh = j // 2, j % 2
                    with nc.allow_low_precision(reason="bf16 out"):
                        nc.vector.tensor_mul(
                            yT_sb[64 * hh:64 * (hh + 1), m,
                                  512 * n4:512 * (n4 + 1)],
                            st["yh"][j][0:64, :], rbcs[j][:])

            def pv():
                n4, c, qo, stp = pvq.pop(0)
                if c == 0:
                    open_window(n4)
                nch = 4 * (n4 + 1)
                for j in range(4):
                    nc.tensor.matmul(
                        psy[j][0:65, qo:512],
                        lhsT=v4[:, c, j, :],
                        rhs=stp[:, j, qo:512],
                        start=(c == 0), stop=(c == nch - 1))
                if c == nch - 1:
                    close_window(n4)

            while si < len(stream):
                emit_chunk()
                npop = 2 if (len(pvq) > 4 and si % 2 == 0) else 1
                for _ in range(min(npop, len(pvq))):
                    pv()
            while pvq:
                pv()

            if DEBUG:
                for j in range(4):
                    nc.sync.dma_start(out=yh_d[:, 512 * j:512 * (j + 1)],
                                      in_=win[3]["yh"][j][:])

            # ---------------- phase 3: out-projection ----------------
            ph3ps = [P01[:, 0, :], P01[:, 1, :], P23[:, 0, :], P23[:, 1, :]]
            for mo in range(8):
                oc = outp.tile([128, T], BF16, tag="oc", name="oc")
                for pn in range(NT4):
                    if mo == 0 and pn == 3:
                        # win3 normalize: broadcast 1/den across partitions
                        # with K=1 matmuls into the now-dead psy banks (the
                        # ~10us serial gpsimd broadcast chain is the tail's
                        # critical path otherwise), then DVE-normalize yT
                        st3 = win[NT4 - 1]
                        for j in range(4):
                            nc.tensor.matmul(psy[j][0:64, :], lhsT=ones64[:],
                                             rhs=st3["rr"][j][:],
                                             start=True, stop=True)
                        for j in range(4):
                            m3, hh3 = j // 2, j % 2
                            with nc.allow_low_precision(reason="bf16 out"):
                                nc.vector.tensor_mul(
                                    yT_sb[64 * hh3:64 * (hh3 + 1), m3,
                                          1536:2048],
                                    st3["yh"][j][0:64, :], psy[j][0:64, :])
                    for cc in range(2):
                        nc.tensor.matmul(
                            ph3ps[pn],
                            lhsT=wp_sb[:, cc, 128 * mo:128 * (mo + 1)],
                            rhs=yT_sb[:, cc, 512 * pn:512 * (pn + 1)],
                            start=(cc == 0), stop=(cc == 1))
                    dst = oc[:, 512 * pn:512 * (pn + 1)]
                    if pn % 2 == 0:
                        nc.scalar.activation(dst, ph3ps[pn], AF.Copy)
                    else:
                        with nc.allow_low_precision(reason="bf16 out"):
                            nc.vector.tensor_copy(dst, ph3ps[pn])
                eng = nc.sync if mo % 2 == 0 else nc.gpsimd
                eng.dma_start(out=yout[128 * mo:128 * (mo + 1), :], in_=oc[:])

            if DEBUG:
                nc.sync.dma_start(out=qT_d, in_=qT_sb[:].rearrange("p m t -> p (m t)"))
                nc.sync.dma_start(out=kT_d, in_=kT_sb[:].rearrange("p m t -> p (m t)"))
                nc.sync.dma_start(out=v4_d, in_=v4[:].rearrange("p a b c -> p (a b c)"))
                nc.sync.dma_start(out=yT_d, in_=yT_sb[:].rearrange("p m t -> p (m t)"))
                nc.sync.dma_start(out=st_d, in_=last_stp[0][:].rearrange("p a b -> p (a b)"))

    nc.compile()
    return nc


def _bf16():
    import ml_dtypes
    return ml_dtypes.bfloat16


def _rearr(w2):
    # [KC*128, M] -> [128, KC*M] so the SBUF load is one contiguous DMA
    m = w2.shape[1]
    return np.ascontiguousarray(
        w2.reshape(KC, 128, m).transpose(1, 0, 2).reshape(128, KC * m))


def kernel(x, Wq, bq, Wk, bk, Wv, bv, Wp, bp):
    global _PROG, LAST_RESULTS
    from concourse.bass_utils import run_bass_kernel_spmd

    x = np.asarray(x, np.float32)
    Wq = np.asarray(Wq, np.float32)
    bq = np.asarray(bq, np.float32)
    Wk = np.asarray(Wk, np.float32)
    bk = np.asarray(bk, np.float32)
    Wv = np.asarray(Wv, np.float32)
    bv = np.asarray(bv, np.float32)
    Wp = np.asarray(Wp, np.float32)
    bp = np.asarray(bp, np.float32)

    if _PROG is None:
        _PROG = _build()
    nc = _PROG

    scale = np.float32(1.0 / np.sqrt(HD))
    k_i = np.arange(128)[:, None]
    q_i = np.arange(128)[None, :]
    tri = (q_i >= k_i).astype(np.float32)           # [k, q] lower-tri in S^T
    mask4 = np.broadcast_to(tri[:, None, :], (128, 4, 128)).reshape(128, 512)
    mask_b = np.ascontiguousarray(mask4).astype(_bf16())

    in_maps = []
    for r in range(NCORES):
        tp, dp = r % TPG, r // TPG
        sl = slice(DH * tp, DH * (tp + 1))
        in_maps.append({
            "xT": np.ascontiguousarray(x[dp].T).astype(_bf16()),
            "wq4": _rearr((Wq[sl] * scale).T).astype(_bf16()),
            "wk4": _rearr(Wk[sl].T).astype(_bf16()),
            "wv4": _rearr(Wv[sl].T).astype(_bf16()),
            "wp4": np.ascontiguousarray(
                Wp[:, sl].T.reshape(2, 128, C).transpose(1, 0, 2)
                .reshape(128, 2 * C)).astype(_bf16()),
            "bq2": np.ascontiguousarray((bq[sl] * scale).reshape(2, 128).T),
            "bk2": np.ascontiguousarray(bk[sl].reshape(2, 128).T),
            "bv1": bv[sl].reshape(1, DH).copy(),
            "mask_d": mask_b,
        })

    res = run_bass_kernel_spmd(nc, in_maps, core_ids=list(range(NCORES)),
                               trace=TRACE)
    LAST_RESULTS = res

    out = np.empty((B, T, C), np.float32)
    for dp in range(B):
        acc = res.results[TPG * dp]["yout"].astype(np.float32)
        for tp in range(1, TPG):
            acc += res.results[TPG * dp + tp]["yout"].astype(np.float32)
        out[dp] = acc.T + bp
    return out


# revision 36
# speedup vs baseline: 1.0616x; 1.0330x over previous
"""Causal self-attention (B=2, T=2048, C=1024, 16 heads) on 8 trn2 NeuronCores.

Sharding: tensor-parallel over heads (4-way) x data-parallel over batch (2-way).
Core r handles batch dp = r // 4 and heads [4*tp, 4*tp+4) where tp = r % 4.

v4 design (ACT-engine-bound problem: softmax exp is ~58us/core minimum).
All PSUM lives in ONE pool of four double-bank [128,2,512] tiles (P01, P23,
P45, P67) that are reused phase to phase; the tile framework's subtile WAR
tracking sequences the reuse:
  P01: m=0 q psum (n0,n1) -> score tile for heads j=0,1 -> phase-3 psum
  P23: m=0 q psum (n2,n3) -> score tile for heads j=2,3 -> phase-3 psum
  P45: m=0 k psum (n0,n1) -> m=1 q/k psum (n-outer)   -> PV accum j=0,1
  P67: m=0 k psum (n2,n3) -> warm-up + v psum          -> PV accum j=2,3
Phase 2 ping-pongs scores/exp between P01 and P23: scores of chunk i+1
overwrite P01 while the exp of chunk i's P23 half still runs, so the ACT
exp stream never waits on the PE (the ACT->PE->ACT semaphore round trip is
~1us; with a single score tile it lands on every op).
The exp is one [128,2,*] ACTIVATE per half (80 total); the causal mask is a
single [128,4,128] bf16 multiply on the diagonal block only.  PV uses the
[v|1] ones-column trick (M=65): psum row 64 accumulates the softmax
denominator.  Window close = DVE copy [65,512] + den staged to partition 0
(reciprocal_approx_fast mis-reads partition-offset inputs) + reciprocal +
gpsimd broadcast + DVE normalize into yT.  Phase 3 is pn-outer/cc-inner so
psum->SBUF bf16 copies (alternating Scalar/Vector) pipeline with the MMs;
output is DMA'd as bf16 row-blocks (host sums the 4 TP partials + bias).
q/k biases are DVE tensor_scalar ops (keeps ACT free for exp); PE warm-up
matmuls + a tiny warm exp at t0 hold the HAM clock gate at 8/8 and preload
the exp table during the initial DMA window.
"""

import numpy as np

B, T, C = 2, 2048, 1024
NH, HD = 16, 64
NCORES, TPG = 8, 4          # 4-way tensor parallel x 2-way data parallel
HPC = NH // TPG             # heads per core (4)
DH = HPC * HD               # per-core head channels (256)
KC = C // 128               # contraction chunks over C (8)
NT4 = T // 512              # 512-wide q windows (4)
NT = T // 128               # 128-wide k/t tiles (16)

_PROG = None
TRACE = False
DEBUG = False
LAST_RESULTS = None


def _build():
    import concourse.bacc as bacc
    import concourse.mybir as mybir
    from concourse import tile

    F32 = mybir.dt.float32
    BF16 = mybir.dt.bfloat16
    AF = mybir.ActivationFunctionType

    nc = bacc.Bacc("TRN2", target_bir_lowering=False, debug=False,
                   num_devices=NCORES)

    xT = nc.dram_tensor("xT", [C, T], BF16, kind="ExternalInput").ap()
    wq4 = nc.dram_tensor("wq4", [128, KC * DH], BF16, kind="ExternalInput").ap()
    wk4 = nc.dram_tensor("wk4", [128, KC * DH], BF16, kind="ExternalInput").ap()
    wv4 = nc.dram_tensor("wv4", [128, KC * DH], BF16, kind="ExternalInput").ap()
    wp4 = nc.dram_tensor("wp4", [128, 2 * C], BF16, kind="ExternalInput").ap()
    bq2 = nc.dram_tensor("bq2", [128, 2], F32, kind="ExternalInput").ap()
    bk2 = nc.dram_tensor("bk2", [128, 2], F32, kind="ExternalInput").ap()
    bv1 = nc.dram_tensor("bv1", [1, DH], F32, kind="ExternalInput").ap()
    mask_d = nc.dram_tensor("mask_d", [128, 4 * 128], BF16, kind="ExternalInput").ap()
    yout = nc.dram_tensor("yout", [C, T], BF16, kind="ExternalOutput").ap()
    if DEBUG:
        qT_d = nc.dram_tensor("qT_d", [128, 2 * T], BF16, kind="ExternalOutput").ap()
        kT_d = nc.dram_tensor("kT_d", [128, 2 * T], BF16, kind="ExternalOutput").ap()
        v4_d = nc.dram_tensor("v4_d", [128, NT * HPC * (HD + 1)], BF16,
                              kind="ExternalOutput").ap()
        yT_d = nc.dram_tensor("yT_d", [128, 2 * T], BF16, kind="ExternalOutput").ap()
        yh_d = nc.dram_tensor("yh_d", [65, 4 * 512], F32, kind="ExternalOutput").ap()
        st_d = nc.dram_tensor("st_d", [128, 4 * 512], BF16, kind="ExternalOutput").ap()

    with tile.TileContext(nc) as tc:
        with tc.tile_pool(name="const", bufs=1) as constp, \
             tc.tile_pool(name="qkv", bufs=1) as qkvp, \
             tc.tile_pool(name="xt", bufs=1) as xtp, \
             tc.tile_pool(name="strip", bufs=15) as stripp, \
             tc.tile_pool(name="rec", bufs=1) as recp, \
             tc.tile_pool(name="outp", bufs=2) as outp, \
             tc.tile_pool(name="ps", bufs=1, space="PSUM") as psp:

            wq_sb = constp.tile([128, KC, DH], BF16)
            wk_sb = constp.tile([128, KC, DH], BF16)
            wv_sb = constp.tile([128, KC, DH], BF16)
            wp_sb = constp.tile([128, 2, C], BF16)
            bq_sb = constp.tile([128, 2], F32)
            bk_sb = constp.tile([128, 2], F32)
            bv_sb = constp.tile([1, DH], F32)
            bv_bc = constp.tile([128, DH], F32)
            mask_sb = constp.tile([128, 4, 128], BF16)
            warm_in = constp.tile([128, 96], BF16)
            warm_out = constp.tile([1, 64], F32)
            ones64 = constp.tile([1, 64], F32)

            qT_sb = qkvp.tile([128, 2, T], BF16)   # [64*(h%2)+d, h//2, t]
            kT_sb = qkvp.tile([128, 2, T], BF16)
            v4 = qkvp.tile([128, NT, HPC, HD + 1], BF16)  # [t%128, t//128, h, d|1]
            yT_sb = qkvp.tile([128, 2, T], BF16)
            xT_sb = xtp.tile([128, KC, T], BF16)

            P01 = psp.tile([128, 2, 512], F32, tag="p01", name="P01")
            P23 = psp.tile([128, 2, 512], F32, tag="p23", name="P23")
            P45 = psp.tile([128, 2, 512], F32, tag="p45", name="P45")
            P67 = psp.tile([128, 2, 512], F32, tag="p67", name="P67")
            # m0 psum slice for (w, n):  q -> P01/P23, k -> P45/P67
            m0ps = {(0, 0): P01[:, 0, :], (0, 1): P01[:, 1, :],
                    (0, 2): P23[:, 0, :], (0, 3): P23[:, 1, :],
                    (1, 0): P45[:, 0, :], (1, 1): P45[:, 1, :],
                    (1, 2): P67[:, 0, :], (1, 3): P67[:, 1, :]}
            psy = [P45[:, 0, :], P45[:, 1, :], P67[:, 0, :], P67[:, 1, :]]

            # ---------------- t0: DMA issues, warm-ups ----------------
            nc.gpsimd.memset(warm_in[:], 0.0078125)
            nc.gpsimd.memset(ones64[:], 1.0)
            nc.gpsimd.dma_start(out=wq_sb[:].rearrange("p c m -> p (c m)"), in_=wq4)
            nc.gpsimd.dma_start(out=wk_sb[:].rearrange("p c m -> p (c m)"), in_=wk4)
            nc.gpsimd.dma_start(out=bq_sb[:], in_=bq2)
            nc.gpsimd.dma_start(out=bk_sb[:], in_=bk2)
            nc.gpsimd.dma_start(out=wv_sb[:].rearrange("p c m -> p (c m)"), in_=wv4)
            nc.gpsimd.dma_start(out=bv_sb[:], in_=bv1)
            nc.gpsimd.dma_start(out=mask_sb[:].rearrange("p a b -> p (a b)"), in_=mask_d)
            nc.gpsimd.dma_start(out=wp_sb[:].rearrange("p c m -> p (c m)"), in_=wp4)
            nc.gpsimd.partition_broadcast(bv_bc[:], bv_sb[:])
            nc.vector.memset(v4[:], 1.0)  # ones column survives; rest overwritten
            xTr = xT.rearrange("(c p) t -> p c t", p=128)
            for c in range(KC):
                nc.sync.dma_start(out=xT_sb[:, c, :], in_=xTr[:, c, :])
            # preload the exp table set while ACT is idle
            nc.scalar.activation(warm_out[:], warm_in[0:1, 0:64], AF.Exp)
            # keep the PE busy through the DMA window so HAM sits at 8/8
            for _ in range(52):
                nc.tensor.matmul(P67[0:96, 1, 0:96], lhsT=warm_in[:, 0:96],
                                 rhs=warm_in[:, 0:96], start=True, stop=True)

            def bias_dve(w, n, m):
                b_sb = bq_sb if w == 0 else bk_sb
                dst = (qT_sb if w == 0 else kT_sb)[:, m, 512 * n:512 * (n + 1)]
                src = m0ps[(w, n)] if m == 0 else P45[:, w, :]
                with nc.allow_low_precision(reason="bf16 out"):
                    nc.vector.tensor_scalar_add(dst, src, b_sb[:, m:m + 1])

            # ---------------- phase 1a: q/k m=0 (c-outer, 8 banks) ----------
            for c in range(KC):
                for w, w_sb in ((0, wq_sb), (1, wk_sb)):
                    for n in range(NT4):
                        nc.tensor.matmul(
                            m0ps[(w, n)], lhsT=w_sb[:, c, 0:128],
                            rhs=xT_sb[:, c, 512 * n:512 * (n + 1)],
                            start=(c == 0), stop=(c == KC - 1))
            # biases that gate the m=1 psum (P45) and the score tiles (P01/P23)
            for w, n in ((1, 0), (1, 1), (0, 0), (0, 1), (0, 2), (0, 3)):
                bias_dve(w, n, 0)

            # ---------------- phases 1b + 2 ----------------
            stream = [(n4, c) for n4 in range(NT4) for c in range(4 * (n4 + 1))]
            pvq = []
            normq = []
            close_si = [0]
            win = {}
            si = 0  # next stream chunk to emit
            last_stp = [None]

            def emit_chunk():
                nonlocal si
                n4, c = stream[si]
                o = c - 4 * n4
                qo = 128 * o if o > 0 else 0
                stp = stripp.tile([128, 4, 512], BF16, tag="stp", name="stp")
                for m, Ps in ((0, P01), (1, P23)):
                    for hh in range(2):
                        nc.tensor.matmul(
                            Ps[:, hh, qo:512],
                            lhsT=kT_sb[64 * hh:64 * (hh + 1), m,
                                       128 * c:128 * (c + 1)],
                            rhs=qT_sb[64 * hh:64 * (hh + 1), m,
                                      512 * n4 + qo:512 * (n4 + 1)],
                            start=True, stop=True, tile_position=(64 * hh, 0))
                    # per-half exp: scores of the next chunk overwrite P01
                    # while this chunk's P23 exp still runs
                    nc.scalar.activation(stp[:, 2 * m:2 * m + 2, qo:512],
                                         Ps[:, :, qo:512], AF.Exp)
                if o >= 0:
                    with nc.allow_low_precision(reason="0/1 mask"):
                        nc.vector.tensor_mul(stp[:, :, qo:qo + 128],
                                             stp[:, :, qo:qo + 128],
                                             mask_sb[:])
                pvq.append((n4, c, qo, stp))
                last_stp[0] = stp
                si += 1

            def emit_m1_half(n, half):
                for c in range(4 * half, 4 * half + 4):
                    for w, w_sb in ((0, wq_sb), (1, wk_sb)):
                        nc.tensor.matmul(
                            P45[:, w, :], lhsT=w_sb[:, c, 128:256],
                            rhs=xT_sb[:, c, 512 * n:512 * (n + 1)],
                            start=(c == 0), stop=(c == KC - 1))
                if half == 1:
                    for w in range(2):
                        bias_dve(w, n, 1)

            def emit_v_tile(t):
                vp = P67[:, 0, 0:DH]
                for c in range(KC):
                    nc.tensor.matmul(
                        vp, lhsT=xT_sb[:, c, 128 * t:128 * (t + 1)],
                        rhs=wv_sb[:, c, :], start=(c == 0), stop=(c == KC - 1))
                with nc.allow_low_precision(reason="f32r bits == f32 bits"):
                    nc.vector.tensor_add(
                        v4[:, t, :, 0:HD],
                        vp.rearrange("p (h d) -> p h d", h=HPC),
                        bv_bc[:].rearrange("p (h d) -> p h d", h=HPC))

            # m=1 n-blocks, v tiles, and early chunks interleaved so the exp
            # stream starts while projections still run on the PE.
            emit_m1_half(0, 0)
            emit_m1_half(0, 1)
            emit_chunk()                      # 0 (needs only n0 q/k)
            emit_m1_half(1, 0)
            emit_m1_half(1, 1)
            emit_chunk()                      # 1
            bias_dve(1, 2, 0)                 # frees P67 slice 0 for v psum
            bias_dve(1, 3, 0)
            emit_chunk()                      # 2
            emit_m1_half(2, 0)
            emit_m1_half(2, 1)
            emit_chunk()                      # 3
            for t in (0, 1):
                emit_v_tile(t)
            emit_chunk()                      # 4
            for t in (2, 3):
                emit_v_tile(t)
            emit_chunk()                      # 5
            emit_m1_half(3, 0)
            emit_m1_half(3, 1)
            emit_chunk()                      # 6
            for t in (4, 5):
                emit_v_tile(t)
            emit_chunk()                      # 7
            for t in (6, 7):
                emit_v_tile(t)
            emit_chunk()                      # 8
            for t in (8, 9):
                emit_v_tile(t)
            emit_chunk()                      # 9
            for t in (10, 11):
                emit_v_tile(t)
            emit_chunk()                      # 10
            for t in (12, 13):
                emit_v_tile(t)
            emit_chunk()                      # 11
            for t in (14, 15):
                emit_v_tile(t)
            emit_chunk()                      # 12

            def open_window(n4):
                win[n4] = dict(
                    yh=[recp.tile([65, 512], F32, tag=f"yh{j}", name="yh")
                        for j in range(4)],
                    dn=[recp.tile([1, 512], F32, tag=f"dn{j}", name="dn")
                        for j in range(4)],
                    rr=[recp.tile([1, 512], F32, tag=f"rr{j}", name="rr")
                        for j in range(4)],
                )

            def close_window(n4):
                st = win[n4]
                rbcs = []
                for j in range(4):
                    with nc.allow_low_precision(reason="f32 bits"):
                        nc.vector.tensor_copy(st["yh"][j][:], psy[j][0:65, :])
                    # recip_approx_fast mis-reads partition-offset inputs --
                    # stage the den row at partition 0 first
                    with nc.allow_low_precision(reason="f32 bits"):
                        nc.vector.tensor_copy(st["dn"][j][:],
                                              st["yh"][j][64:65, :])
                    nc.vector.reciprocal_approx_fast(st["rr"][j][:],
                                                     st["dn"][j][:])
                    if n4 == NT4 - 1:
                        continue  # win3 normalize runs in phase 3 (PE bcast)
                    # emit the broadcast right away so gpsimd overlaps the
                    # remaining DVE copies (the muls stay in a second loop so
                    # they don't block the DVE queue on gpsimd latency)
                    rbc = recp.tile([64, 512], F32, tag="rbc", bufs=4,
                                    name="rbc")
                    nc.gpsimd.partition_broadcast(rbc[:], st["rr"][j][:])
                    rbcs.append(rbc)
                if n4 == NT4 - 1:
                    return
                for j in range(4):
                    m, hh = j // 2, j % 2
                    with nc.allow_low_precision(reason="bf16 out"):
                        nc.vector.tensor_mul(
                            yT_sb[64 * hh:64 * (hh + 1), m,
                                  512 * n4:512 * (n4 + 1)],
                            st["yh"][j][0:64, :], rbcs[j][:])

            def pv():
                n4, c, qo, stp = pvq.pop(0)
                if c == 0:
                    open_window(n4)
                nch = 4 * (n4 + 1)
                for j in range(4):
                    nc.tensor.matmul(
                        psy[j][0:65, qo:512],
                        lhsT=v4[:, c, j, :],
                        rhs=stp[:, j, qo:512],
                        start=(c == 0), stop=(c == nch - 1))
                if c == nch - 1:
                    close_window(n4)

            while si < len(stream):
                emit_chunk()
                npop = 2 if (len(pvq) > 4 and si % 2 == 0) else 1
                for _ in range(min(npop, len(pvq))):
                    pv()
            while pvq:
                pv()

            if DEBUG:
                for j in range(4):
                    nc.sync.dma_start(out=yh_d[:, 512 * j:512 * (j + 1)],
                                      in_=win[3]["yh"][j][:])

            # ---------------- phase 3: out-projection ----------------
            ph3ps = [P01[:, 0, :], P01[:, 1, :], P23[:, 0, :], P23[:, 1, :]]
            for mo in range(8):
                oc = outp.tile([128, T], BF16, tag="oc", name="oc")
                for pn in range(NT4):
                    if mo == 0 and pn == 3:
                        # win3 normalize: broadcast 1/den across partitions
                        # with K=1 matmuls into the now-dead psy banks (the
                        # ~10us serial gpsimd broadcast chain is the tail's
                        # critical path otherwise), then DVE-normalize yT
                        st3 = win[NT4 - 1]
                        for j in range(4):
                            nc.tensor.matmul(psy[j][0:64, :], lhsT=ones64[:],
                                             rhs=st3["rr"][j][:],
                                             start=True, stop=True)
                        # keep-warm filler: the pn3 MMs below wait on the
                        # recip->bcast->mul chain (~4us); without PE work in
                        # that window HAM re-throttles and the remaining 7
                        # mo-blocks run at half clock.  pn0's psum is already
                        # copied out, so scribbling there delays nothing.
                        for _ in range(14):
                            nc.tensor.matmul(P01[0:96, 0, :],
                                             lhsT=warm_in[:, 0:96],
                                             rhs=xT_sb[:, 0, 0:512],
                                             start=True, stop=True)
                        for j in range(4):
                            m3, hh3 = j // 2, j % 2
                            with nc.allow_low_precision(reason="bf16 out"):
                                nc.vector.tensor_mul(
                                    yT_sb[64 * hh3:64 * (hh3 + 1), m3,
                                          1536:2048],
                                    st3["yh"][j][0:64, :], psy[j][0:64, :])
                    for cc in range(2):
                        nc.tensor.matmul(
                            ph3ps[pn],
                            lhsT=wp_sb[:, cc, 128 * mo:128 * (mo + 1)],
                            rhs=yT_sb[:, cc, 512 * pn:512 * (pn + 1)],
                            start=(cc == 0), stop=(cc == 1))
                    dst = oc[:, 512 * pn:512 * (pn + 1)]
                    if pn % 2 == 0:
                        nc.scalar.activation(dst, ph3ps[pn], AF.Copy)
                    else:
                        with nc.allow_low_precision(reason="bf16 out"):
                            nc.vector.tensor_copy(dst, ph3ps[pn])
                eng = nc.sync if mo % 2 == 0 else nc.gpsimd
                eng.dma_start(out=yout[128 * mo:128 * (mo + 1), :], in_=oc[:])

            if DEBUG:
                nc.sync.dma_start(out=qT_d, in_=qT_sb[:].rearrange("p m t -> p (m t)"))
                nc.sync.dma_start(out=kT_d, in_=kT_sb[:].rearrange("p m t -> p (m t)"))
                nc.sync.dma_start(out=v4_d, in_=v4[:].rearrange("p a b c -> p (a b c)"))
                nc.sync.dma_start(out=yT_d, in_=yT_sb[:].rearrange("p m t -> p (m t)"))
                nc.sync.dma_start(out=st_d, in_=last_stp[0][:].rearrange("p a b -> p (a b)"))

    nc.compile()
    return nc


def _bf16():
    import ml_dtypes
    return ml_dtypes.bfloat16


def _rearr(w2):
    # [KC*128, M] -> [128, KC*M] so the SBUF load is one contiguous DMA
    m = w2.shape[1]
    return np.ascontiguousarray(
        w2.reshape(KC, 128, m).transpose(1, 0, 2).reshape(128, KC * m))


def kernel(x, Wq, bq, Wk, bk, Wv, bv, Wp, bp):
    global _PROG, LAST_RESULTS
    from concourse.bass_utils import run_bass_kernel_spmd

    x = np.asarray(x, np.float32)
    Wq = np.asarray(Wq, np.float32)
    bq = np.asarray(bq, np.float32)
    Wk = np.asarray(Wk, np.float32)
    bk = np.asarray(bk, np.float32)
    Wv = np.asarray(Wv, np.float32)
    bv = np.asarray(bv, np.float32)
    Wp = np.asarray(Wp, np.float32)
    bp = np.asarray(bp, np.float32)

    if _PROG is None:
        _PROG = _build()
    nc = _PROG

    scale = np.float32(1.0 / np.sqrt(HD))
    k_i = np.arange(128)[:, None]
    q_i = np.arange(128)[None, :]
    tri = (q_i >= k_i).astype(np.float32)           # [k, q] lower-tri in S^T
    mask4 = np.broadcast_to(tri[:, None, :], (128, 4, 128)).reshape(128, 512)
    mask_b = np.ascontiguousarray(mask4).astype(_bf16())

    in_maps = []
    for r in range(NCORES):
        tp, dp = r % TPG, r // TPG
        sl = slice(DH * tp, DH * (tp + 1))
        in_maps.append({
            "xT": np.ascontiguousarray(x[dp].T).astype(_bf16()),
            "wq4": _rearr((Wq[sl] * scale).T).astype(_bf16()),
            "wk4": _rearr(Wk[sl].T).astype(_bf16()),
            "wv4": _rearr(Wv[sl].T).astype(_bf16()),
            "wp4": np.ascontiguousarray(
                Wp[:, sl].T.reshape(2, 128, C).transpose(1, 0, 2)
                .reshape(128, 2 * C)).astype(_bf16()),
            "bq2": np.ascontiguousarray((bq[sl] * scale).reshape(2, 128).T),
            "bk2": np.ascontiguousarray(bk[sl].reshape(2, 128).T),
            "bv1": bv[sl].reshape(1, DH).copy(),
            "mask_d": mask_b,
        })

    res = run_bass_kernel_spmd(nc, in_maps, core_ids=list(range(NCORES)),
                               trace=TRACE)
    LAST_RESULTS = res

    out = np.empty((B, T, C), np.float32)
    for dp in range(B):
        acc = res.results[TPG * dp]["yout"].astype(np.float32)
        for tp in range(1, TPG):
            acc += res.results[TPG * dp + tp]["yout"].astype(np.float32)
        out[dp] = acc.T + bp
    return out
